# revision 1
# baseline (speedup 1.0000x reference)
"""Causal self-attention (QK-RMSNorm + RoPE) on 8 Trainium2 NeuronCores.

Problem: x[2,2048,2048], Wq/Wk/Wv/Wo [2048,2048], 16 heads, head_dim 128.

Sharding: core c handles batch b=c//4 and head group g=c%4 (4 heads,
model cols [512g:512g+512)).  QKV projections are computed from the
host-pre-transposed xT (contraction dim on partitions).  Attention uses
transposed scores (eT = exp(scale * kT_blk.T @ qT_chunk)), so the AV
matmul (lhsT=v, rhs=eT) directly yields the transposed attention output
yT[d, i] that o_proj consumes.  The softmax denominator comes from a
ones-lhsT matmul over eT, its reciprocal is broadcast across partitions
with a K=1 PE matmul.  Per-batch groups of 4 cores AllGather their
yT head shards, then each core computes a 512-column slice of the
output projection (y = attn @ Wo.T) in transposed layout.  The host
de-transposes and concatenates.  No AllReduce is needed.

Matmuls run in fp32r (full PE rate, ~1.5e-4 rel err).
"""

import math
from contextlib import ExitStack

import numpy as np

import concourse.bass as bass
import concourse.bacc as bacc
import concourse.tile as tile
from concourse import mybir
from concourse.bass_utils import run_bass_kernel_spmd
from concourse.masks import make_identity

P = 128
D = 2048
S = 2048
HD = 128              # head dim
NHL = 4               # heads per core
GW = NHL * HD         # 512, per-core width of head group
CT = D // P           # 16 contraction tiles
ICH = 4               # i-chunks of 512 positions
NCORES = 8
F32 = mybir.dt.float32
F16 = mybir.dt.float16
F32R = mybir.dt.float32r
SCALE = 1.0 / math.sqrt(HD)
EPS = 1.1920928955078125e-07
MASK_NEG = -30000.0

_program_cache = {}


def build_program():
    if "nc" in _program_cache:
        return _program_cache["nc"]

    nc = bacc.Bacc("TRN2", target_bir_lowering=False, debug=False, num_devices=NCORES)

    xt_in = nc.dram_tensor("xt", [D, S], F16, kind="ExternalInput")
    wq_in = nc.dram_tensor("wq", [D, GW], F16, kind="ExternalInput")
    wk_in = nc.dram_tensor("wk", [D, GW], F16, kind="ExternalInput")
    wv_in = nc.dram_tensor("wv", [D, GW], F16, kind="ExternalInput")
    wo_in = nc.dram_tensor("wo", [D, GW], F16, kind="ExternalInput")
    cos_in = nc.dram_tensor("cos", [S, HD // 2], F16, kind="ExternalInput")
    sin_in = nc.dram_tensor("sin", [S, HD // 2], F16, kind="ExternalInput")
    mask_in = nc.dram_tensor("maskt", [4, P, 512], F16, kind="ExternalInput")
    roff_in = nc.dram_tensor("roff", [1, 2], mybir.dt.uint32, kind="ExternalInput")
    yt_out = nc.dram_tensor("yt_out", [GW, S], F32, kind="ExternalOutput")

    with tile.TileContext(nc) as tc:
        with ExitStack() as ctx:
            const = ctx.enter_context(tc.tile_pool(name="const", bufs=1))
            dram = ctx.enter_context(tc.tile_pool(name="dram", bufs=1, space="DRAM"))

            ident = const.tile([P, P], F16, name="ident")
            make_identity(nc, ident)
            eps_t = const.tile([P, 1], F32, name="eps_t")
            nc.vector.memset(eps_t[:], EPS)
            zero_t = const.tile([P, 1], F32, name="zero_t")
            nc.vector.memset(zero_t[:], 0.0)
            neg1_t = const.tile([P, 1], F32, name="neg1_t")
            nc.vector.memset(neg1_t[:], -1.0)
            ones_f = const.tile([P, P], F32, name="ones_f")
            nc.vector.memset(ones_f[:], 1.0)
            ones2 = const.tile([P, 2], F16, name="ones2")
            nc.scalar.copy(ones2[:], ones_f[:, 0:2])
            ones_row = const.tile([1, P], F32R, name="ones_row")
            nc.scalar.copy(ones_row[:], ones_f[0:1, :])

            cos_sb = const.tile([P, CT, HD // 2], F16, name="cos_sb")
            nc.sync.dma_start(out=cos_sb[:], in_=cos_in.ap().rearrange("(a p) f -> p a f", p=P))
            sin_sb = const.tile([P, CT, HD // 2], F16, name="sin_sb")
            nc.sync.dma_start(out=sin_sb[:], in_=sin_in.ap().rearrange("(a p) f -> p a f", p=P))

            qt_d = dram.tile([GW, S], F16, name="qt_d")
            kt_d = dram.tile([GW, S], F16, name="kt_d")
            yt_ics = [dram.tile([GW, 512], F16, name=f"yt_ic{i}") for i in range(ICH)]
            ag_ics = [
                dram.tile([NCORES * GW, 512], F16, name=f"ag_ic{i}", addr_space="Shared")
                for i in range(ICH)
            ]

            # wo pool created up-front so phase A can prefetch into it
            wopool = ctx.enter_context(tc.tile_pool(name="wopool", bufs=1))
            wo_sb = wopool.tile([P, CT, GW], F16, name="wo_sb")

            # ---------------- Phase A: Q and K ----------------
            with ExitStack() as pha:
                wpool = pha.enter_context(tc.tile_pool(name="wpool", bufs=1))
                xt_pool = pha.enter_context(tc.tile_pool(name="xt_pool", bufs=2))
                proj_ps = pha.enter_context(tc.tile_pool(name="proj_ps", bufs=2, space="PSUM"))
                tp_ps = pha.enter_context(tc.tile_pool(name="tp_ps", bufs=2, space="PSUM"))
                rope = pha.enter_context(tc.tile_pool(name="rope", bufs=2))
                stat = pha.enter_context(tc.tile_pool(name="stat", bufs=2))
                evq = pha.enter_context(tc.tile_pool(name="evq", bufs=3))

                wq_sb = wpool.tile([P, CT, GW], F16, name="wq_sb")
                wk_sb = wpool.tile([P, CT, GW], F16, name="wk_sb")
                # prefetch O weights late in phase A (pool lives in outer scope)
                for ica in range(8):
                    xt_ch = xt_pool.tile([P, CT, 256], F16, name=f"xt_ch{ica}", tag="xt")
                    for ct in range(CT):
                        if ica == 0:
                            # interleave weight-tile and first-chunk loads so the
                            # first accumulation groups unblock ct-by-ct
                            nc.sync.dma_start(out=wq_sb[:, ct, :], in_=wq_in[ct * P:(ct + 1) * P, :])
                        nc.sync.dma_start(
                            out=xt_ch[:, ct, :],
                            in_=xt_in[ct * P:(ct + 1) * P, ica * 256:(ica + 1) * 256],
                        )
                        if ica == 0:
                            nc.sync.dma_start(out=wk_sb[:, ct, :], in_=wk_in[ct * P:(ct + 1) * P, :])
                        if ica == 1:
                            nc.sync.dma_start(out=wo_sb[:, ct, :], in_=wo_in[ct * P:(ct + 1) * P, :])
                    for ib in range(2):
                        ibg = ica * 2 + ib         # global i-block
                        i0 = ibg * P
                        for wsb, dst in ((wq_sb, qt_d), (wk_sb, kt_d)):
                            ps = proj_ps.tile([P, GW], F32, name=f"ps{ibg}", tag="proj")
                            for ct in range(CT):
                                nc.tensor.matmul(
                                    ps[:],
                                    xt_ch[:, ct, ib * P:(ib + 1) * P],
                                    wsb[:, ct, :],
                                    start=(ct == 0),
                                    stop=(ct == CT - 1),
                                )
                            # rms norm stats + rope, all row-wise
                            qs = rope.tile([P, GW], F16, name=f"qs{ibg}", tag="qs")
                            nc.scalar.copy(qs[:], ps[:])
                            sq = rope.tile([P, GW], F16, name=f"sq{ibg}", tag="sq")
                            nc.vector.tensor_mul(sq[:], qs[:], qs[:])
                            rstd = stat.tile([P, NHL], F32, name=f"rstd{ibg}", tag="rstd")
                            for h in range(NHL):
                                nc.vector.reduce_sum(
                                    rstd[:, h:h + 1],
                                    sq[:, h * HD:(h + 1) * HD],
                                    axis=mybir.AxisListType.X,
                                )
                            nc.scalar.activation(
                                rstd[:], rstd[:],
                                mybir.ActivationFunctionType.Sqrt,
                                bias=eps_t[:], scale=1.0 / HD,
                            )
                            nc.vector.reciprocal(rstd[:], rstd[:])

                            q3 = qs[:].rearrange("p (h d) -> p h d", h=NHL)
                            qr = rope.tile([P, GW], F16, name=f"qr{ibg}", tag="qr")
                            qr3 = qr[:].rearrange("p (h d) -> p h d", h=NHL)
                            tmp = rope.tile([P, NHL, HD // 2], F16, name=f"tmp{ibg}", tag="tmp")
                            cosB = cos_sb[:, ibg:ibg + 1, :].broadcast_to((P, NHL, HD // 2))
                            sinB = sin_sb[:, ibg:ibg + 1, :].broadcast_to((P, NHL, HD // 2))
                            h1 = q3[:, :, 0:HD // 2]
                            h2 = q3[:, :, HD // 2:HD]
                            # r1 = q1*cos + q2*sin ; r2 = q2*cos - q1*sin
                            nc.vector.tensor_mul(qr3[:, :, 0:HD // 2], h1, cosB)
                            nc.vector.tensor_mul(tmp[:], h2, sinB)
                            nc.vector.tensor_add(qr3[:, :, 0:HD // 2], qr3[:, :, 0:HD // 2], tmp[:])
                            nc.vector.tensor_mul(qr3[:, :, HD // 2:HD], h2, cosB)
                            nc.vector.tensor_mul(tmp[:], h1, sinB)
                            nc.vector.tensor_sub(
                                qr3[:, :, HD // 2:HD], qr3[:, :, HD // 2:HD], tmp[:]
                            )
                            for h in range(NHL):
                                nc.vector.tensor_scalar_mul(
                                    qr[:, h * HD:(h + 1) * HD],
                                    qr[:, h * HD:(h + 1) * HD],
                                    rstd[:, h:h + 1],
                                )
                            # transpose per head, evict to DRAM
                            for h in range(NHL):
                                tp = tp_ps.tile([P, P], F16, name=f"tp{ibg}_{h}", tag="tp")
                                nc.tensor.transpose(tp[:], qr[:, h * HD:(h + 1) * HD], ident[:])
                                qtt = evq.tile([P, P], F16, name=f"qtt{ibg}_{h}", tag="qtt")
                                nc.scalar.copy(qtt[:], tp[:])
                                nc.scalar.dma_start(
                                    out=dst[h * P:(h + 1) * P, i0:i0 + P], in_=qtt[:]
                                )

            # wv/wo/v pools: wo persists to phase D; wv+v live A..B
            # (their DMAs are issued inside phase A for prefetch)

            # ---------------- Phase A2: V ----------------
            with ExitStack() as phab:
                # v + mask live A2..B; freed before phase D
                vpool = phab.enter_context(tc.tile_pool(name="vpool", bufs=1))
                v_sb = vpool.tile([P, CT, GW], F16, name="v_sb")
                maskp = phab.enter_context(tc.tile_pool(name="maskp", bufs=1))
                mask_sb = maskp.tile([P, 4, 512], F16, name="mask_sb")
                nc.sync.dma_start(out=mask_sb[:], in_=mask_in.ap().rearrange("t p f -> p t f"))

                with ExitStack() as phv:
                    wvpool = phv.enter_context(tc.tile_pool(name="wvpool", bufs=1))
                    xt2_pool = phv.enter_context(tc.tile_pool(name="xt2_pool", bufs=2))
                    v_ps = phv.enter_context(tc.tile_pool(name="v_ps", bufs=2, space="PSUM"))
                    wv_sb = wvpool.tile([P, CT, GW], F16, name="wv_sb")
                    for ct in range(CT):
                        nc.sync.dma_start(out=wv_sb[:, ct, :], in_=wv_in[ct * P:(ct + 1) * P, :])

                    for ic2 in range(8):
                        xt_ch = xt2_pool.tile([P, CT, 256], F16, name=f"x2_{ic2}", tag="xt2")
                        for ct in range(CT):
                            nc.sync.dma_start(
                                out=xt_ch[:, ct, :],
                                in_=xt_in[ct * P:(ct + 1) * P, ic2 * 256:(ic2 + 1) * 256],
                            )
                        for ib in range(2):
                            jb = ic2 * 2 + ib
                            ps = v_ps.tile([P, GW], F32, name=f"vps{jb}", tag="vps")
                            for ct in range(CT):
                                nc.tensor.matmul(
                                    ps[:],
                                    xt_ch[:, ct, ib * P:(ib + 1) * P],
                                    wv_sb[:, ct, :],
                                    start=(ct == 0),
                                    stop=(ct == CT - 1),
                                )
                            nc.scalar.copy(v_sb[:, jb, :], ps[:])

                # ---------------- Phase B: attention, i-chunk outer ----------------
                with ExitStack() as phb:
                    kt_pool = phb.enter_context(tc.tile_pool(name="kt_pool", bufs=2))
                    qt_pool = phb.enter_context(tc.tile_pool(name="qt_pool", bufs=2))
                    et_pool = phb.enter_context(tc.tile_pool(name="et_pool", bufs=4))
                    s_ps = phb.enter_context(tc.tile_pool(name="s_ps", bufs=3, space="PSUM"))
                    acc_ps = phb.enter_context(tc.tile_pool(name="acc_ps", bufs=2, space="PSUM"))
                    bc_psp = phb.enter_context(tc.tile_pool(name="bc_psp", bufs=1, space="PSUM"))
                    bsmall = phb.enter_context(tc.tile_pool(name="bsmall", bufs=2))

                    for ic in range(ICH):
                        njb = 4 * ic + 4
                        kt_t = kt_pool.tile([P, NHL, 2048], F16, name=f"kt{ic}", tag="kt")
                        qt_t = qt_pool.tile([P, NHL, 512], F16, name=f"qt{ic}", tag="qt")
                        for h in range(NHL):
                            nc.sync.dma_start(
                                out=kt_t[:, h, 0:njb * P],
                                in_=kt_d[h * P:(h + 1) * P, 0:njb * P],
                            )
                            nc.sync.dma_start(
                                out=qt_t[:, h, :],
                                in_=qt_d[h * P:(h + 1) * P, ic * 512:(ic + 1) * 512],
                            )
                        for h in range(NHL):
                            yt_ps = acc_ps.tile([P, 512], F32, name=f"yt{h}_{ic}", tag="yt")
                            den_ps = acc_ps.tile([2, 512], F32, name=f"den{h}_{ic}", tag="den")
                            for jb in range(njb):
                                sp = s_ps.tile([P, 512], F32, name=f"s{h}_{ic}_{jb}", tag="s")
                                nc.tensor.matmul(
                                    sp[:],
                                    kt_t[:, h, jb * P:(jb + 1) * P],
                                    qt_t[:, h, :],
                                    start=True, stop=True,
                                )
                                et = et_pool.tile([P, 512], F16, name=f"et{h}_{ic}_{jb}", tag="et")
                                nc.scalar.activation(
                                    et[:], sp[:],
                                    mybir.ActivationFunctionType.Exp,
                                    bias=neg1_t[:], scale=SCALE,
                                )
                                t = jb - 4 * ic
                                if t >= 0:
                                    nc.vector.tensor_mul(et[:], et[:], mask_sb[:, t, :])
                                nc.tensor.matmul(
                                    yt_ps[:],
                                    v_sb[:, jb, h * HD:(h + 1) * HD],
                                    et[:],
                                    start=(jb == 0), stop=(jb == njb - 1),
                                )
                                nc.tensor.matmul(
                                    den_ps[:],
                                    ones2[:],
                                    et[:],
                                    start=(jb == 0), stop=(jb == njb - 1),
                                )
                            rden = bsmall.tile([1, 512], F32, name=f"rd{h}_{ic}", tag="rden")
                            nc.vector.reciprocal(rden[:], den_ps[0:1, :])
                            rden_r = bsmall.tile([1, 512], F32R, name=f"rdr{h}_{ic}", tag="rdenr")
                            nc.scalar.copy(rden_r[:], rden[:])
                            bc_ps = bc_psp.tile([P, 512], F32, name=f"bc{h}_{ic}", tag="bc")
                            nc.tensor.matmul(bc_ps[:], ones_row[:], rden_r[:], start=True, stop=True)
                            bc_sb = bsmall.tile([P, 512], F32, name=f"bcs{h}_{ic}", tag="bcs")
                            nc.vector.tensor_copy(bc_sb[:], bc_ps[:])
                            yt_sb = bsmall.tile([P, 512], F16, name=f"yts{h}_{ic}", tag="yts")
                            nc.vector.tensor_mul(yt_sb[:], yt_ps[:], bc_sb[:])
                            nc.gpsimd.dma_start(
                                out=yt_ics[ic][h * P:(h + 1) * P, :],
                                in_=yt_sb[:],
                            )
                        # per-chunk AllGather fires as soon as chunk ic is written
                        nc.gpsimd.collective_compute(
                            "AllGather",
                            mybir.AluOpType.bypass,
                            replica_groups=[list(range(NCORES))],
                            ins=[yt_ics[ic][:].opt()],
                            outs=[ag_ics[ic][:].opt()],
                        )

            # ---------------- Phase D: o_proj (column shard), pipelined per chunk ----------------
            with ExitStack() as phd:
                ag_pool = phd.enter_context(tc.tile_pool(name="ag_pool", bufs=2))
                d_ps = phd.enter_context(tc.tile_pool(name="d_ps", bufs=2, space="PSUM"))
                ev2 = phd.enter_context(tc.tile_pool(name="ev2", bufs=3))
                roffp = phd.enter_context(tc.tile_pool(name="roffp", bufs=1))

                roff_sb = roffp.tile([1, 2], mybir.dt.uint32, name="roff_sb")
                nc.sync.dma_start(out=roff_sb[:], in_=roff_in[:, :])
                roff_reg = nc.alloc_registers()
                nc.regs_load(roff_reg, roff_sb[0:1, 0:1])
                rv = nc.snap(roff_reg, donate=True)

                for icc in range(ICH):
                    ag_ch = ag_pool.tile([P, CT, 512], F16, name=f"ag{icc}", tag="ag")
                    nc.sync.dma_start(
                        out=ag_ch[:],
                        in_=ag_ics[icc][bass.ds(rv, D), :]
                            .rearrange("(t p) f -> p t f", p=P),
                    )
                    for oc in range(4):
                        y_ps = d_ps.tile([P, 512], F32, name=f"yp{icc}_{oc}", tag="yp")
                        for mt in range(CT):
                            nc.tensor.matmul(
                                y_ps[:],
                                wo_sb[:, mt, oc * P:(oc + 1) * P],
                                ag_ch[:, mt, :],
                                start=(mt == 0), stop=(mt == CT - 1),
                            )
                        y_sb = ev2.tile([P, 512], F32, name=f"ysb{icc}_{oc}", tag="ysb")
                        nc.scalar.copy(y_sb[:], y_ps[:])
                        nc.scalar.dma_start(
                            out=yt_out[oc * P:(oc + 1) * P, icc * 512:(icc + 1) * 512],
                            in_=y_sb[:],
                        )

    nc.compile()
    _program_cache["nc"] = nc
    return nc


def _rope_tables():
    inv_freq = 1.0 / (10000.0 ** (np.arange(0, HD, 2, dtype=np.float32) / HD))
    pos = np.arange(S, dtype=np.float32)
    freqs = np.outer(pos, inv_freq).astype(np.float32)
    return np.cos(freqs).astype(np.float16), np.sin(freqs).astype(np.float16)


def _mask_tiles():
    m = np.zeros((4, P, 512), dtype=np.float16)
    jj = np.arange(P)[:, None]
    ii = np.arange(512)[None, :]
    for t in range(4):
        m[t] = np.where(t * P + jj > ii, 0.0, 1.0)
    return m


def make_in_maps(x, Wq, Wk, Wv, Wo):
    x = np.asarray(x, dtype=np.float32)
    cos, sin = _rope_tables()
    maskt = _mask_tiles()
    wqT = np.ascontiguousarray(np.asarray(Wq, dtype=np.float32).T.astype(np.float16))
    wkT = np.ascontiguousarray(np.asarray(Wk, dtype=np.float32).T.astype(np.float16))
    wvT = np.ascontiguousarray(np.asarray(Wv, dtype=np.float32).T.astype(np.float16))
    woT = np.ascontiguousarray(np.asarray(Wo, dtype=np.float32).T.astype(np.float16))
    xts = [np.ascontiguousarray(x[b].T.astype(np.float16)) for b in range(2)]
    in_maps = []
    for c in range(NCORES):
        b, g = c // 4, c % 4
        sl = slice(g * GW, (g + 1) * GW)
        in_maps.append({
            "roff": np.array([[b * D, 0]], dtype=np.uint32),
            "xt": xts[b],
            "wq": np.ascontiguousarray(wqT[:, sl]),
            "wk": np.ascontiguousarray(wkT[:, sl]),
            "wv": np.ascontiguousarray(wvT[:, sl]),
            "wo": np.ascontiguousarray(woT[:, sl]),
            "cos": cos,
            "sin": sin,
            "maskt": maskt,
        })
    return in_maps


def assemble_output(results):
    y = np.empty((2, S, D), dtype=np.float32)
    for c in range(NCORES):
        b, g = c // 4, c % 4
        y[b][:, g * GW:(g + 1) * GW] = results[c]["yt_out"].T
    return y


def kernel(x, Wq, Wk, Wv, Wo):
    nc = build_program()
    in_maps = make_in_maps(x, Wq, Wk, Wv, Wo)
    res = run_bass_kernel_spmd(nc, in_maps, core_ids=list(range(NCORES)))
    return assemble_output(res.results)



# revision 15
# speedup vs baseline: 1.0644x; 1.0644x over previous
"""Causal self-attention (QK-RMSNorm + RoPE) on 8 Trainium2 NeuronCores.

Problem: x[2,2048,2048], Wq/Wk/Wv/Wo [2048,2048], 16 heads, head_dim 128.

Sharding: core c handles batch b=c//4 and head group g=c%4 (4 heads,
model cols [512g:512g+512)).

Single fused pipeline, one pass over x per core:
- Q/K are projected directly into transposed [head_dim, tokens] layout
  by making the weight tile the stationary matmul operand (no PE
  transposes, no DRAM roundtrip).  V is projected in [tokens, cols]
  layout for the AV matmul.
- RMS-norm uses a ones[128,128] matmul to produce the per-token sum of
  squares broadcast across all partitions in one shot; normalization is
  a single DVE divide.  RoPE runs on 64-partition halves against a
  transposed cos/sin table.
- Attention per 512-token chunk uses transposed scores
  (eT = exp(scale*kT.T@qT - 1)); the softmax denominator is accumulated
  on the vector engine (csum += eT) and turned into a broadcast
  denominator with one ones-matmul per (head, chunk); yt = yt_acc / den.
- Per-chunk AllGather over the 4 cores of each batch (not all 8), then
  each core computes a 512-row slice of yT = Wo @ yt_full, interleaved
  at two chunks of lag so collectives hide under compute.
"""

import math
from contextlib import ExitStack

import numpy as np

import concourse.bass as bass
import concourse.bacc as bacc
import concourse.tile as tile
from concourse import mybir
from concourse.bass_utils import run_bass_kernel_spmd

P = 128
D = 2048
S = 2048
HD = 128              # head dim
NHL = 4               # heads per core
GW = NHL * HD         # 512, per-core width of head group
CT = D // P           # 16 contraction tiles
NTCH = 4              # token chunks of 512
NCORES = 8
F32 = mybir.dt.float32
F16 = mybir.dt.float16
F32R = mybir.dt.float32r
SCALE = 1.0 / math.sqrt(HD)
EPS = 1.1920928955078125e-07

_program_cache = {}


def build_program():
    if "nc" in _program_cache:
        return _program_cache["nc"]

    nc = bacc.Bacc("TRN2", target_bir_lowering=False, debug=False, num_devices=NCORES)

    xt_in = nc.dram_tensor("xt", [D, S], F16, kind="ExternalInput")
    wq_in = nc.dram_tensor("wq", [D, GW], F16, kind="ExternalInput")
    wk_in = nc.dram_tensor("wk", [D, GW], F16, kind="ExternalInput")
    wv_in = nc.dram_tensor("wv", [D, GW], F16, kind="ExternalInput")
    wo_in = nc.dram_tensor("wo", [D, GW], F16, kind="ExternalInput")
    cs_in = nc.dram_tensor("cs", [P, 2, S], F16, kind="ExternalInput")
    mask_in = nc.dram_tensor("maskt", [4, P, 512], F16, kind="ExternalInput")
    yt_out = nc.dram_tensor("yt_out", [GW, S], F32, kind="ExternalOutput")

    with tile.TileContext(nc) as tc:
        with ExitStack() as ctx:
            const = ctx.enter_context(tc.tile_pool(name="const", bufs=1))
            dram = ctx.enter_context(tc.tile_pool(name="dram", bufs=1, space="DRAM"))

            eps_t = const.tile([P, 1], F32, name="eps_t")
            nc.vector.memset(eps_t[:], EPS)
            neg1_t = const.tile([P, 1], F32, name="neg1_t")
            nc.vector.memset(neg1_t[:], -1.0)
            ones_h = const.tile([P, P], F16, name="ones_h")
            nc.vector.memset(ones_h[:], 1.0)
            ones_r = const.tile([P, P], F32R, name="ones_r")
            nc.scalar.copy(ones_r[:], ones_h[:])

            # plane 0: cos duplicated on both partition halves; plane 1:
            # +sin on rows 0..63, -sin on rows 64..127 (rope sign folded)
            cs_sb = const.tile([P, 2, S], F16, name="cs_sb")
            nc.sync.dma_start(out=cs_sb[:], in_=cs_in[:, :, :])
            mask_sb = const.tile([P, 4, 512], F16, name="mask_sb")
            nc.sync.dma_start(out=mask_sb[:], in_=mask_in.ap().rearrange("t p f -> p t f"))

            yt_ics = [dram.tile([GW, 512], F16, name=f"yt_ic{i}") for i in range(NTCH)]
            ag_ics = [
                dram.tile([4 * GW, 512], F16, name=f"ag_ic{i}")
                for i in range(NTCH)
            ]

            # persistent SBUF
            wpool = ctx.enter_context(tc.tile_pool(name="wpool", bufs=1))
            wq_sb = wpool.tile([P, CT, GW], F16, name="wq_sb")
            wk_sb = wpool.tile([P, CT, GW], F16, name="wk_sb")
            wv_sb = wpool.tile([P, CT, GW], F16, name="wv_sb")
            wo_sb = wpool.tile([P, CT, GW], F16, name="wo_sb")
            qkv = ctx.enter_context(tc.tile_pool(name="qkv", bufs=1))
            qt_sb = qkv.tile([P, NHL, S], F16, name="qt_sb")
            kt_sb = qkv.tile([P, NHL, S], F16, name="kt_sb")
            v_sb = qkv.tile([P, CT, GW], F16, name="v_sb")

            # streaming pools (x in 256-token half-chunks)
            xtp = ctx.enter_context(tc.tile_pool(name="xtp", bufs=3))
            rawp = ctx.enter_context(tc.tile_pool(name="rawp", bufs=2))
            sqp = ctx.enter_context(tc.tile_pool(name="sqp", bufs=3))
            nrmp = ctx.enter_context(tc.tile_pool(name="nrmp", bufs=2))
            qsp = ctx.enter_context(tc.tile_pool(name="qsp", bufs=2))
            mp = ctx.enter_context(tc.tile_pool(name="mp", bufs=2))
            etp = ctx.enter_context(tc.tile_pool(name="etp", bufs=4))
            csp = ctx.enter_context(tc.tile_pool(name="csp", bufs=2))
            denp = ctx.enter_context(tc.tile_pool(name="denp", bufs=2))
            ytsp = ctx.enter_context(tc.tile_pool(name="ytsp", bufs=2))
            agp = ctx.enter_context(tc.tile_pool(name="agp", bufs=2))
            ysp = ctx.enter_context(tc.tile_pool(name="ysp", bufs=2))

            # PSUM: 2+2+2+2 = 8 banks
            proj_ps = ctx.enter_context(tc.tile_pool(name="proj_ps", bufs=2, space="PSUM"))
            s_ps = ctx.enter_context(tc.tile_pool(name="s_ps", bufs=2, space="PSUM"))
            yt_ps = ctx.enter_context(tc.tile_pool(name="yt_ps", bufs=2, space="PSUM"))
            bc_ps = ctx.enter_context(tc.tile_pool(name="bc_ps", bufs=2, space="PSUM"))

            # weight loads: wq per-ct on sync (interleaved with x chunk 0
            # below); wk/wv/wo as single rearranged DMAs on scalar
            nc.scalar.dma_start(
                out=wk_sb[:], in_=wk_in.ap().rearrange("(a p) f -> p a f", p=P))
            nc.scalar.dma_start(
                out=wv_sb[:], in_=wv_in.ap().rearrange("(a p) f -> p a f", p=P))
            nc.scalar.dma_start(
                out=wo_sb[:], in_=wo_in.ap().rearrange("(a p) f -> p a f", p=P))

            def emit_oproj(icc):
                ag_a = agp.tile([P, 8, 512], F16, name=f"ag_a{icc}", tag="ag")
                ag_b = agp.tile([P, 8, 512], F16, name=f"ag_b{icc}", tag="ag")
                for half, agt in ((0, ag_a), (1, ag_b)):
                    for m8 in range(8):
                        mt = half * 8 + m8
                        nc.sync.dma_start(
                            out=agt[:, m8, :],
                            in_=ag_ics[icc][mt * P:(mt + 1) * P, :],
                        )
                for oc in range(4):
                    yp = proj_ps.tile([P, 512], F32, name=f"yp{icc}_{oc}", tag="proj")
                    for mt in range(CT):
                        agt = ag_a if mt < 8 else ag_b
                        nc.tensor.matmul(
                            yp[:],
                            wo_sb[:, mt, oc * P:(oc + 1) * P],
                            agt[:, mt % 8, :],
                            start=(mt == 0), stop=(mt == CT - 1),
                        )
                    y_sb = ysp.tile([P, 512], F32, name=f"ysb{icc}_{oc}", tag="ysb")
                    nc.scalar.copy(y_sb[:], yp[:])
                    nc.scalar.dma_start(
                        out=yt_out[oc * P:(oc + 1) * P, icc * 512:(icc + 1) * 512],
                        in_=y_sb[:],
                    )

            for tch in range(NTCH):
                tc0 = tch * 512
                for half in range(2):
                    hc0 = tc0 + half * 256
                    xt_ch = xtp.tile(
                        [P, CT, 256], F16, name=f"xt{tch}_{half}", tag="xt")
                    for ct in range(CT):
                        if tch == 0 and half == 0:
                            nc.sync.dma_start(
                                out=wq_sb[:, ct, :], in_=wq_in[ct * P:(ct + 1) * P, :])
                        nc.sync.dma_start(
                            out=xt_ch[:, ct, :],
                            in_=xt_in[ct * P:(ct + 1) * P, hc0:hc0 + 256],
                        )

                    # ---- Q then K: transposed projection + rms-norm + rope ----
                    for wsb, dst, tag in ((wq_sb, qt_sb, "q"), (wk_sb, kt_sb, "k")):
                        raw4 = rawp.tile(
                            [P, NHL, 256], F16, name=f"{tag}raw{tch}_{half}", tag="raw")
                        nrm4 = nrmp.tile(
                            [P, NHL, 256], F16, name=f"{tag}nrm{tch}_{half}", tag="nrm")
                        sqs = []
                        for h in range(NHL):
                            ps = proj_ps.tile(
                                [P, 256], F32, name=f"{tag}ps{tch}_{half}_{h}",
                                tag="proj")
                            for ct in range(CT):
                                nc.tensor.matmul(
                                    ps[:],
                                    wsb[:, ct, h * P:(h + 1) * P],
                                    xt_ch[:, ct, :],
                                    start=(ct == 0), stop=(ct == CT - 1),
                                )
                            nc.scalar.copy(raw4[:, h, :], ps[:])
                            sq = sqp.tile(
                                [P, 256], F16, name=f"{tag}sq{tch}_{half}_{h}",
                                tag="sq")
                            nc.vector.tensor_mul(sq[:], raw4[:, h, :], raw4[:, h, :])
                            sqs.append(sq)
                        # partition-swapped copy of raw4 (q2 on rows 0..63)
                        qs4 = qsp.tile(
                            [P, NHL, 256], F16, name=f"{tag}qs{tch}_{half}", tag="qs")
                        nc.sync.dma_start(out=qs4[0:64, :, :], in_=raw4[64:128, :, :])
                        nc.sync.dma_start(out=qs4[64:128, :, :], in_=raw4[0:64, :, :])
                        for h in range(NHL):
                            ssum = bc_ps.tile(
                                [P, 256], F32, name=f"{tag}ss{tch}_{half}_{h}",
                                tag="bc")
                            nc.tensor.matmul(
                                ssum[:], ones_h[:], sqs[h][:], start=True, stop=True)
                            # rstd = exp(-0.5*ln(ms+eps)) — Ln and Exp share
                            # one ACT table set, so no table switches
                            lnt = sqp.tile(
                                [P, 256], F16, name=f"{tag}ln{tch}_{half}_{h}",
                                tag="lnt")
                            nc.scalar.activation(
                                lnt[:], ssum[:],
                                mybir.ActivationFunctionType.Ln,
                                bias=eps_t[:], scale=1.0 / HD,
                            )
                            nc.scalar.activation(
                                nrm4[:, h, :], lnt[:],
                                mybir.ActivationFunctionType.Exp,
                                scale=-0.5,
                            )
                        # rope: m1 = raw*cos_dup; m2 = swapped*sin_signed;
                        # dst = (m1 + m2) * rstd
                        cosB = cs_sb[:, 0:1, hc0:hc0 + 256].broadcast_to((P, NHL, 256))
                        sinB = cs_sb[:, 1:2, hc0:hc0 + 256].broadcast_to((P, NHL, 256))
                        m1 = mp.tile(
                            [P, NHL, 256], F16, name=f"{tag}m1{tch}_{half}", tag="m1")
                        m2 = mp.tile(
                            [P, NHL, 256], F16, name=f"{tag}m2{tch}_{half}", tag="m2")
                        nc.vector.tensor_mul(m1[:], raw4[:], cosB)
                        nc.vector.tensor_mul(m2[:], qs4[:], sinB)
                        nc.vector.tensor_add(m1[:], m1[:], m2[:])
                        nc.vector.tensor_mul(
                            dst[:, :, hc0:hc0 + 256], m1[:], nrm4[:])

                    # ---- V: row-layout projection ----
                    for ib in range(2):
                        jb = tch * 4 + half * 2 + ib
                        ps = proj_ps.tile([P, GW], F32, name=f"vps{jb}", tag="proj")
                        for ct in range(CT):
                            nc.tensor.matmul(
                                ps[:],
                                xt_ch[:, ct, ib * P:(ib + 1) * P],
                                wv_sb[:, ct, :],
                                start=(ct == 0), stop=(ct == CT - 1),
                            )
                        nc.scalar.copy(v_sb[:, jb, :], ps[:])

                # ---- attention for chunk tch ----
                njb = 4 * (tch + 1)
                for h in range(NHL):
                    ytp = yt_ps.tile([P, 512], F32, name=f"yt{tch}_{h}", tag="yt")
                    csum = csp.tile([P, 512], F32R, name=f"cs{tch}_{h}", tag="cs")
                    ets = [None] * njb
                    for jb in range(njb):
                        sp = s_ps.tile([P, 512], F32, name=f"s{tch}_{h}_{jb}", tag="s")
                        nc.tensor.matmul(
                            sp[:],
                            kt_sb[:, h, jb * P:(jb + 1) * P],
                            qt_sb[:, h, tc0:tc0 + 512],
                            start=True, stop=True,
                        )
                        et = etp.tile([P, 512], F16, name=f"et{tch}_{h}_{jb}", tag="et")
                        nc.scalar.activation(
                            et[:], sp[:],
                            mybir.ActivationFunctionType.Exp,
                            bias=neg1_t[:], scale=SCALE,
                        )
                        t = jb - 4 * tch
                        if t >= 0:
                            nc.vector.tensor_mul(et[:], et[:], mask_sb[:, t, :])
                        if jb == 0:
                            nc.vector.tensor_scalar_mul(csum[:], et[:], 1.0)
                        else:
                            nc.vector.tensor_add(csum[:], csum[:], et[:])
                        ets[jb] = et
                        # AV lags score by one tile so PE never waits on exp
                        if jb >= 1:
                            nc.tensor.matmul(
                                ytp[:],
                                v_sb[:, jb - 1, h * HD:(h + 1) * HD],
                                ets[jb - 1][:],
                                start=(jb - 1 == 0), stop=False,
                            )
                    nc.tensor.matmul(
                        ytp[:],
                        v_sb[:, njb - 1, h * HD:(h + 1) * HD],
                        ets[njb - 1][:],
                        start=(njb == 1), stop=True,
                    )
                    den = bc_ps.tile([P, 512], F32, name=f"den{tch}_{h}", tag="bc")
                    nc.tensor.matmul(
                        den[:], ones_r[:], csum[:], start=True, stop=True)
                    # rden = exp(-ln(den)) on the scalar engine (same ACT
                    # table set as the softmax exp)
                    lnd = denp.tile([P, 512], F32, name=f"lnd{tch}_{h}", tag="lnd", bufs=1)
                    nc.scalar.activation(
                        lnd[:], den[:], mybir.ActivationFunctionType.Ln)
                    rden = denp.tile([P, 512], F32, name=f"rdn{tch}_{h}", tag="rden")
                    nc.scalar.activation(
                        rden[:], lnd[:], mybir.ActivationFunctionType.Exp,
                        scale=-1.0)
                    yt_sb = ytsp.tile([P, 512], F16, name=f"yts{tch}_{h}", tag="yts")
                    nc.vector.tensor_mul(yt_sb[:], ytp[:], rden[:])
                    nc.gpsimd.dma_start(
                        out=yt_ics[tch][h * P:(h + 1) * P, :], in_=yt_sb[:])

                nc.gpsimd.collective_compute(
                    "AllGather",
                    mybir.AluOpType.bypass,
                    replica_groups=[[0, 1, 2, 3], [4, 5, 6, 7]],
                    ins=[yt_ics[tch][:].opt()],
                    outs=[ag_ics[tch][:].opt()],
                )
                if tch >= 2:
                    emit_oproj(tch - 2)

            emit_oproj(2)
            emit_oproj(3)

    nc.compile()
    _program_cache["nc"] = nc
    return nc


def _rope_tables():
    inv_freq = 1.0 / (10000.0 ** (np.arange(0, HD, 2, dtype=np.float32) / HD))
    pos = np.arange(S, dtype=np.float32)
    freqs = np.outer(pos, inv_freq).astype(np.float32)  # [S, 64]
    cosT = np.cos(freqs).T  # [64, S]
    sinT = np.sin(freqs).T
    cs = np.empty((P, 2, S), dtype=np.float16)
    cs[0:64, 0] = cosT
    cs[64:128, 0] = cosT
    cs[0:64, 1] = sinT
    cs[64:128, 1] = -sinT
    return cs


def _mask_tiles():
    m = np.zeros((4, P, 512), dtype=np.float16)
    jj = np.arange(P)[:, None]
    ii = np.arange(512)[None, :]
    for t in range(4):
        m[t] = np.where(t * P + jj > ii, 0.0, 1.0)
    return m


def make_in_maps(x, Wq, Wk, Wv, Wo):
    x = np.asarray(x, dtype=np.float32)
    cs = _rope_tables()
    maskt = _mask_tiles()
    wqT = np.ascontiguousarray(np.asarray(Wq, dtype=np.float32).T.astype(np.float16))
    wkT = np.ascontiguousarray(np.asarray(Wk, dtype=np.float32).T.astype(np.float16))
    wvT = np.ascontiguousarray(np.asarray(Wv, dtype=np.float32).T.astype(np.float16))
    woT = np.ascontiguousarray(np.asarray(Wo, dtype=np.float32).T.astype(np.float16))
    xts = [np.ascontiguousarray(x[b].T.astype(np.float16)) for b in range(2)]
    in_maps = []
    for c in range(NCORES):
        b, g = c // 4, c % 4
        sl = slice(g * GW, (g + 1) * GW)
        in_maps.append({
            "xt": xts[b],
            "wq": np.ascontiguousarray(wqT[:, sl]),
            "wk": np.ascontiguousarray(wkT[:, sl]),
            "wv": np.ascontiguousarray(wvT[:, sl]),
            "wo": np.ascontiguousarray(woT[:, sl]),
            "cs": cs,
            "maskt": maskt,
        })
    return in_maps


def assemble_output(results):
    y = np.empty((2, S, D), dtype=np.float32)
    for c in range(NCORES):
        b, g = c // 4, c % 4
        y[b][:, g * GW:(g + 1) * GW] = results[c]["yt_out"].T
    return y


def kernel(x, Wq, Wk, Wv, Wo):
    nc = build_program()
    in_maps = make_in_maps(x, Wq, Wk, Wv, Wo)
    res = run_bass_kernel_spmd(nc, in_maps, core_ids=list(range(NCORES)))
    return assemble_output(res.results)


# revision 19
# speedup vs baseline: 1.2291x; 1.1548x over previous
"""Causal self-attention (QK-RMSNorm + RoPE) on 8 Trainium2 NeuronCores.

Problem: x[2,2048,2048], Wq/Wk/Wv/Wo [2048,2048], 16 heads, head_dim 128.

Sharding: core c handles batch b=c//4 and head group g=c%4 (4 heads,
model cols [512g:512g+512)).

Single fused pipeline, one pass over x per core:
- Q/K are projected directly into transposed [head_dim, tokens] layout
  by making the weight tile the stationary matmul operand (no PE
  transposes, no DRAM roundtrip).  V is projected in [tokens, cols]
  layout for the AV matmul.
- RMS-norm uses a ones[128,128] matmul to produce the per-token sum of
  squares broadcast across all partitions in one shot; normalization is
  a single DVE divide.  RoPE runs on 64-partition halves against a
  transposed cos/sin table.
- Attention per 512-token chunk uses transposed scores
  (eT = exp(scale*kT.T@qT - 1)); the softmax denominator is accumulated
  on the vector engine (csum += eT) and turned into a broadcast
  denominator with one ones-matmul per (head, chunk); yt = yt_acc / den.
- Per-chunk AllGather over the 4 cores of each batch (not all 8), then
  each core computes a 512-row slice of yT = Wo @ yt_full, interleaved
  at two chunks of lag so collectives hide under compute.
"""

import math
from contextlib import ExitStack

import numpy as np

import bass_rust as _bass_rust

import concourse.bass as bass
import concourse.bacc as bacc
import concourse.tile as tile
from concourse import mybir
from concourse.bass_utils import run_bass_kernel_spmd
from concourse.hw_specs import get_activation_tables

P = 128
D = 2048
S = 2048
HD = 128              # head dim
NHL = 4               # heads per core
GW = NHL * HD         # 512, per-core width of head group
CT = D // P           # 16 contraction tiles
NTCH = 4              # token chunks of 512
NCORES = 8
F32 = mybir.dt.float32
F16 = mybir.dt.float16
F32R = mybir.dt.float32r
SCALE = 1.0 / math.sqrt(HD)
EPS = 1.1920928955078125e-07

_program_cache = {}

# All scalar-engine transcendentals here are exp/ln/copy; route every one of
# them to the single ACT table set that contains them all so the table is
# loaded exactly once (the default chooser picks the first covering set per
# function, which alternates sets and costs ~2.7us per switch).
_SET_WITH_ALL = "natural_log_exp_and_others"
_SHARED_FNS = {
    mybir.ActivationFunctionType.Exp,
    mybir.ActivationFunctionType.Ln,
    mybir.ActivationFunctionType.Copy,
}


class _Bacc(bacc.Bacc):
    def insert_act_table_loads(self):
        has_activation = any(
            isinstance(i, mybir.InstActivation)
            for b in self.main_func.blocks
            for i in b.instructions
        )
        if not has_activation:
            return
        tables = []
        for name, fns in get_activation_tables(self.m.arch).items():
            if name != _SET_WITH_ALL:
                fns = fns - _SHARED_FNS
            tables.append((name, fns))
        _bass_rust.insert_act_table_loads(self, tables)


def build_program():
    if "nc" in _program_cache:
        return _program_cache["nc"]

    nc = _Bacc("TRN2", target_bir_lowering=False, debug=False, num_devices=NCORES)

    xt_in = nc.dram_tensor("xt", [D, S], F16, kind="ExternalInput")
    wq_in = nc.dram_tensor("wq", [D, GW], F16, kind="ExternalInput")
    wk_in = nc.dram_tensor("wk", [D, GW], F16, kind="ExternalInput")
    wv_in = nc.dram_tensor("wv", [D, GW], F16, kind="ExternalInput")
    wo_in = nc.dram_tensor("wo", [D, GW], F16, kind="ExternalInput")
    cs_in = nc.dram_tensor("cs", [P, 2, S], F16, kind="ExternalInput")
    mask_in = nc.dram_tensor("maskt", [4, P, 512], F16, kind="ExternalInput")
    yt_out = nc.dram_tensor("yt_out", [GW, S], F32, kind="ExternalOutput")

    with tile.TileContext(nc) as tc:
        with ExitStack() as ctx:
            const = ctx.enter_context(tc.tile_pool(name="const", bufs=1))
            dram = ctx.enter_context(tc.tile_pool(name="dram", bufs=1, space="DRAM"))

            eps_t = const.tile([P, 1], F32, name="eps_t")
            nc.vector.memset(eps_t[:], EPS)
            neg1_t = const.tile([P, 1], F32, name="neg1_t")
            nc.vector.memset(neg1_t[:], -1.0)
            ones_h = const.tile([P, P], F16, name="ones_h")
            nc.vector.memset(ones_h[:], 1.0)

            # plane 0: cos duplicated on both partition halves; plane 1:
            # +sin on rows 0..63, -sin on rows 64..127 (rope sign folded)
            cs_sb = const.tile([P, 2, S], F16, name="cs_sb")
            nc.sync.dma_start(out=cs_sb[:], in_=cs_in[:, :, :])
            mask_sb = const.tile([P, 4, 512], F16, name="mask_sb")
            nc.sync.dma_start(out=mask_sb[:], in_=mask_in.ap().rearrange("t p f -> p t f"))

            yt_ics = [dram.tile([GW, 512], F16, name=f"yt_ic{i}") for i in range(NTCH)]
            ag_ics = [
                dram.tile([4 * GW, 512], F16, name=f"ag_ic{i}")
                for i in range(NTCH)
            ]

            # persistent SBUF
            wpool = ctx.enter_context(tc.tile_pool(name="wpool", bufs=1))
            wq_sb = wpool.tile([P, CT, GW], F16, name="wq_sb")
            wk_sb = wpool.tile([P, CT, GW], F16, name="wk_sb")
            wv_sb = wpool.tile([P, CT, GW], F16, name="wv_sb")
            wo_sb = wpool.tile([P, CT, GW], F16, name="wo_sb")
            qkv = ctx.enter_context(tc.tile_pool(name="qkv", bufs=1))
            qt_sb = qkv.tile([P, NHL, S], F16, name="qt_sb")
            kt_sb = qkv.tile([P, NHL, S], F16, name="kt_sb")
            v_sb = qkv.tile([P, CT, GW], F16, name="v_sb")

            # streaming pools (x in 256-token half-chunks)
            xtp = ctx.enter_context(tc.tile_pool(name="xtp", bufs=3))
            rawp = ctx.enter_context(tc.tile_pool(name="rawp", bufs=2))
            sqp = ctx.enter_context(tc.tile_pool(name="sqp", bufs=3))
            nrmp = ctx.enter_context(tc.tile_pool(name="nrmp", bufs=2))
            qsp = ctx.enter_context(tc.tile_pool(name="qsp", bufs=2))
            mp = ctx.enter_context(tc.tile_pool(name="mp", bufs=2))
            etp = ctx.enter_context(tc.tile_pool(name="etp", bufs=4))
            denp = ctx.enter_context(tc.tile_pool(name="denp", bufs=2))
            ytsp = ctx.enter_context(tc.tile_pool(name="ytsp", bufs=2))
            agp = ctx.enter_context(tc.tile_pool(name="agp", bufs=2))
            ysp = ctx.enter_context(tc.tile_pool(name="ysp", bufs=2))

            # PSUM: 2+2+2+2 = 8 banks
            proj_ps = ctx.enter_context(tc.tile_pool(name="proj_ps", bufs=2, space="PSUM"))
            s_ps = ctx.enter_context(tc.tile_pool(name="s_ps", bufs=2, space="PSUM"))
            yt_ps = ctx.enter_context(tc.tile_pool(name="yt_ps", bufs=2, space="PSUM"))
            bc_ps = ctx.enter_context(tc.tile_pool(name="bc_ps", bufs=2, space="PSUM"))

            # weight loads: wq per-ct on sync (interleaved with x chunk 0
            # below); wk/wv/wo as single rearranged DMAs on scalar
            nc.scalar.dma_start(
                out=wk_sb[:], in_=wk_in.ap().rearrange("(a p) f -> p a f", p=P))
            nc.scalar.dma_start(
                out=wv_sb[:], in_=wv_in.ap().rearrange("(a p) f -> p a f", p=P))
            nc.scalar.dma_start(
                out=wo_sb[:], in_=wo_in.ap().rearrange("(a p) f -> p a f", p=P))

            def emit_oproj(icc):
                ag_a = agp.tile([P, 8, 512], F16, name=f"ag_a{icc}", tag="ag")
                ag_b = agp.tile([P, 8, 512], F16, name=f"ag_b{icc}", tag="ag")
                for half, agt in ((0, ag_a), (1, ag_b)):
                    for m8 in range(8):
                        mt = half * 8 + m8
                        nc.sync.dma_start(
                            out=agt[:, m8, :],
                            in_=ag_ics[icc][mt * P:(mt + 1) * P, :],
                        )
                for oc in range(4):
                    yp = proj_ps.tile([P, 512], F32, name=f"yp{icc}_{oc}", tag="proj")
                    for mt in range(CT):
                        agt = ag_a if mt < 8 else ag_b
                        nc.tensor.matmul(
                            yp[:],
                            wo_sb[:, mt, oc * P:(oc + 1) * P],
                            agt[:, mt % 8, :],
                            start=(mt == 0), stop=(mt == CT - 1),
                        )
                    y_sb = ysp.tile([P, 512], F32, name=f"ysb{icc}_{oc}", tag="ysb")
                    nc.scalar.copy(y_sb[:], yp[:])
                    nc.scalar.dma_start(
                        out=yt_out[oc * P:(oc + 1) * P, icc * 512:(icc + 1) * 512],
                        in_=y_sb[:],
                    )

            for tch in range(NTCH):
                tc0 = tch * 512
                for half in range(2):
                    hc0 = tc0 + half * 256
                    xt_ch = xtp.tile(
                        [P, CT, 256], F16, name=f"xt{tch}_{half}", tag="xt")
                    for ct in range(CT):
                        if tch == 0 and half == 0:
                            nc.sync.dma_start(
                                out=wq_sb[:, ct, :], in_=wq_in[ct * P:(ct + 1) * P, :])
                        nc.sync.dma_start(
                            out=xt_ch[:, ct, :],
                            in_=xt_in[ct * P:(ct + 1) * P, hc0:hc0 + 256],
                        )

                    # ---- Q then K: transposed projection + rms-norm + rope ----
                    for wsb, dst, tag in ((wq_sb, qt_sb, "q"), (wk_sb, kt_sb, "k")):
                        raw4 = rawp.tile(
                            [P, NHL, 256], F16, name=f"{tag}raw{tch}_{half}", tag="raw")
                        nrm4 = nrmp.tile(
                            [P, NHL, 256], F16, name=f"{tag}nrm{tch}_{half}", tag="nrm")
                        sqs = []
                        for h in range(NHL):
                            ps = proj_ps.tile(
                                [P, 256], F32, name=f"{tag}ps{tch}_{half}_{h}",
                                tag="proj")
                            for ct in range(CT):
                                nc.tensor.matmul(
                                    ps[:],
                                    wsb[:, ct, h * P:(h + 1) * P],
                                    xt_ch[:, ct, :],
                                    start=(ct == 0), stop=(ct == CT - 1),
                                )
                            nc.vector.tensor_copy(raw4[:, h, :], ps[:])
                            sq = sqp.tile(
                                [P, 256], F16, name=f"{tag}sq{tch}_{half}_{h}",
                                tag="sq")
                            nc.vector.tensor_mul(sq[:], raw4[:, h, :], raw4[:, h, :])
                            sqs.append(sq)
                        # partition-swapped copy of raw4 (q2 on rows 0..63)
                        qs4 = qsp.tile(
                            [P, NHL, 256], F16, name=f"{tag}qs{tch}_{half}", tag="qs")
                        nc.sync.dma_start(out=qs4[0:64, :, :], in_=raw4[64:128, :, :])
                        nc.sync.dma_start(out=qs4[64:128, :, :], in_=raw4[0:64, :, :])
                        for h in range(NHL):
                            ssum = bc_ps.tile(
                                [P, 256], F32, name=f"{tag}ss{tch}_{half}_{h}",
                                tag="bc")
                            nc.tensor.matmul(
                                ssum[:], ones_h[:], sqs[h][:], start=True, stop=True)
                            # rstd = exp(-0.5*ln(ms+eps)) — Ln and Exp share
                            # one ACT table set, so no table switches
                            lnt = sqp.tile(
                                [P, 256], F16, name=f"{tag}ln{tch}_{half}_{h}",
                                tag="lnt")
                            nc.scalar.activation(
                                lnt[:], ssum[:],
                                mybir.ActivationFunctionType.Ln,
                                bias=eps_t[:], scale=1.0 / HD,
                            )
                            nc.scalar.activation(
                                nrm4[:, h, :], lnt[:],
                                mybir.ActivationFunctionType.Exp,
                                scale=-0.5,
                            )
                        # rope: m1 = raw*cos_dup; m2 = swapped*sin_signed;
                        # dst = (m1 + m2) * rstd
                        cosB = cs_sb[:, 0:1, hc0:hc0 + 256].broadcast_to((P, NHL, 256))
                        sinB = cs_sb[:, 1:2, hc0:hc0 + 256].broadcast_to((P, NHL, 256))
                        m1 = mp.tile(
                            [P, NHL, 256], F16, name=f"{tag}m1{tch}_{half}", tag="m1")
                        m2 = mp.tile(
                            [P, NHL, 256], F16, name=f"{tag}m2{tch}_{half}", tag="m2")
                        nc.vector.tensor_mul(m1[:], raw4[:], cosB)
                        nc.vector.tensor_mul(m2[:], qs4[:], sinB)
                        nc.vector.tensor_add(m1[:], m1[:], m2[:])
                        nc.vector.tensor_mul(
                            dst[:, :, hc0:hc0 + 256], m1[:], nrm4[:])

                    # ---- V: row-layout projection ----
                    for ib in range(2):
                        jb = tch * 4 + half * 2 + ib
                        ps = proj_ps.tile([P, GW], F32, name=f"vps{jb}", tag="proj")
                        for ct in range(CT):
                            nc.tensor.matmul(
                                ps[:],
                                xt_ch[:, ct, ib * P:(ib + 1) * P],
                                wv_sb[:, ct, :],
                                start=(ct == 0), stop=(ct == CT - 1),
                            )
                        nc.vector.tensor_copy(v_sb[:, jb, :], ps[:])

                # ---- attention for chunk tch ----
                njb = 4 * (tch + 1)
                for h in range(NHL):
                    ytp = yt_ps.tile([P, 512], F32, name=f"yt{tch}_{h}", tag="yt")
                    den = bc_ps.tile([P, 512], F32, name=f"den{tch}_{h}", tag="bc")
                    ets = [None] * njb
                    for jb in range(njb):
                        sp = s_ps.tile([P, 512], F32, name=f"s{tch}_{h}_{jb}", tag="s")
                        nc.tensor.matmul(
                            sp[:],
                            kt_sb[:, h, jb * P:(jb + 1) * P],
                            qt_sb[:, h, tc0:tc0 + 512],
                            start=True, stop=True,
                        )
                        et = etp.tile([P, 512], F16, name=f"et{tch}_{h}_{jb}", tag="et")
                        nc.scalar.activation(
                            et[:], sp[:],
                            mybir.ActivationFunctionType.Exp,
                            bias=neg1_t[:], scale=SCALE,
                        )
                        t = jb - 4 * tch
                        if t >= 0:
                            nc.vector.tensor_mul(et[:], et[:], mask_sb[:, t, :])
                        ets[jb] = et
                        # AV and the ones-matmul denominator accumulation lag
                        # the score by one tile so PE never waits on exp
                        if jb >= 1:
                            nc.tensor.matmul(
                                ytp[:],
                                v_sb[:, jb - 1, h * HD:(h + 1) * HD],
                                ets[jb - 1][:],
                                start=(jb - 1 == 0), stop=False,
                            )
                            nc.tensor.matmul(
                                den[:], ones_h[:], ets[jb - 1][:],
                                start=(jb - 1 == 0), stop=False,
                            )
                    nc.tensor.matmul(
                        ytp[:],
                        v_sb[:, njb - 1, h * HD:(h + 1) * HD],
                        ets[njb - 1][:],
                        start=(njb == 1), stop=True,
                    )
                    nc.tensor.matmul(
                        den[:], ones_h[:], ets[njb - 1][:],
                        start=(njb == 1), stop=True,
                    )
                    # rden = exp(-ln(den)) on the scalar engine (same ACT
                    # table set as the softmax exp)
                    lnd = denp.tile([P, 512], F32, name=f"lnd{tch}_{h}", tag="lnd", bufs=1)
                    nc.scalar.activation(
                        lnd[:], den[:], mybir.ActivationFunctionType.Ln)
                    rden = denp.tile([P, 512], F32, name=f"rdn{tch}_{h}", tag="rden")
                    nc.scalar.activation(
                        rden[:], lnd[:], mybir.ActivationFunctionType.Exp,
                        scale=-1.0)
                    yt_sb = ytsp.tile([P, 512], F16, name=f"yts{tch}_{h}", tag="yts")
                    nc.vector.tensor_mul(yt_sb[:], ytp[:], rden[:])
                    nc.gpsimd.dma_start(
                        out=yt_ics[tch][h * P:(h + 1) * P, :], in_=yt_sb[:])

                nc.gpsimd.collective_compute(
                    "AllGather",
                    mybir.AluOpType.bypass,
                    replica_groups=[[0, 1, 2, 3], [4, 5, 6, 7]],
                    ins=[yt_ics[tch][:].opt()],
                    outs=[ag_ics[tch][:].opt()],
                )
                if tch >= 2:
                    emit_oproj(tch - 2)

            emit_oproj(2)
            emit_oproj(3)

    nc.compile()
    _program_cache["nc"] = nc
    return nc


def _rope_tables():
    inv_freq = 1.0 / (10000.0 ** (np.arange(0, HD, 2, dtype=np.float32) / HD))
    pos = np.arange(S, dtype=np.float32)
    freqs = np.outer(pos, inv_freq).astype(np.float32)  # [S, 64]
    cosT = np.cos(freqs).T  # [64, S]
    sinT = np.sin(freqs).T
    cs = np.empty((P, 2, S), dtype=np.float16)
    cs[0:64, 0] = cosT
    cs[64:128, 0] = cosT
    cs[0:64, 1] = sinT
    cs[64:128, 1] = -sinT
    return cs


def _mask_tiles():
    m = np.zeros((4, P, 512), dtype=np.float16)
    jj = np.arange(P)[:, None]
    ii = np.arange(512)[None, :]
    for t in range(4):
        m[t] = np.where(t * P + jj > ii, 0.0, 1.0)
    return m


def make_in_maps(x, Wq, Wk, Wv, Wo):
    x = np.asarray(x, dtype=np.float32)
    cs = _rope_tables()
    maskt = _mask_tiles()
    wqT = np.ascontiguousarray(np.asarray(Wq, dtype=np.float32).T.astype(np.float16))
    wkT = np.ascontiguousarray(np.asarray(Wk, dtype=np.float32).T.astype(np.float16))
    wvT = np.ascontiguousarray(np.asarray(Wv, dtype=np.float32).T.astype(np.float16))
    woT = np.ascontiguousarray(np.asarray(Wo, dtype=np.float32).T.astype(np.float16))
    xts = [np.ascontiguousarray(x[b].T.astype(np.float16)) for b in range(2)]
    in_maps = []
    for c in range(NCORES):
        b, g = c // 4, c % 4
        sl = slice(g * GW, (g + 1) * GW)
        in_maps.append({
            "xt": xts[b],
            "wq": np.ascontiguousarray(wqT[:, sl]),
            "wk": np.ascontiguousarray(wkT[:, sl]),
            "wv": np.ascontiguousarray(wvT[:, sl]),
            "wo": np.ascontiguousarray(woT[:, sl]),
            "cs": cs,
            "maskt": maskt,
        })
    return in_maps


def assemble_output(results):
    y = np.empty((2, S, D), dtype=np.float32)
    for c in range(NCORES):
        b, g = c // 4, c % 4
        y[b][:, g * GW:(g + 1) * GW] = results[c]["yt_out"].T
    return y


def kernel(x, Wq, Wk, Wv, Wo):
    nc = build_program()
    in_maps = make_in_maps(x, Wq, Wk, Wv, Wo)
    res = run_bass_kernel_spmd(nc, in_maps, core_ids=list(range(NCORES)))
    return assemble_output(res.results)
